# revision 2
# baseline (speedup 1.0000x reference)
"""Trainium2 Bass kernel for nn_HarmonicEstimation (topk_masking).

Problem: x [16,1,1025,1024] f32 -> mask [16,1,1025,1024].
Per (batch, t) column over f-bins 1..1024: find top-5 peaks, f0 = min index
among peaks with value > 0.1 (else 0); output column = harmonic-comb mask
that depends ONLY on f0.

Strategy (8 cores, 2 batches/core, no communication):
  - The output column is a pure function of f0, so precompute on host a
    LUT[f0, k] (1025 x 1088-padded f32 rows) kept in DRAM.
  - Load input tiles in natural [f-part, t-free] layout (contiguous DMA),
    transpose to [t-part, f-free] via PE identity matmuls (PSUM), evacuate
    with the scalar engine.
  - DVE InstMax/InstMaxIndex give per-column top-8 values + indices in one
    pass each; tiny ops derive exact f0 (ties resolved like jax.lax.top_k).
    Columns with no peak > 0.1 (or f0 > 1020) map to the all-0.5 LUT row
    via the f0=1024 sentinel.
  - gpsimd dma_gather pulls LUT rows (one 4.25KB row per column) from DRAM
    into [t-part, k-free] SBUF tiles via the DMA engines; PE transposes
    them back to natural [k-part, t-free] tiles which DMA out contiguous.
    (GPSIMD ap_gather was tried first: ~28us per tile on HW - Q7 cores do
    16 partitions serially - so the gather must ride the DMA engines.)
"""

import os
import sys

for _p in ("/opt/trn_rl_repo", "/root/.axon_site/_ro/trn_rl_repo"):
    if os.path.isdir(_p) and _p not in sys.path:
        sys.path.insert(0, _p)

import numpy as np

import concourse.bacc as bacc
import concourse.mybir as mybir
from concourse.bass_utils import run_bass_kernel_spmd
from concourse.tile import TileContext

dt = mybir.dt
Alu = mybir.AluOpType

B = 16          # full batch
NB = 2          # batches per core
NCORES = 8
F = 1025        # freq bins (0..1024)
T = 1024        # time columns
FT = 8          # f tiles of 128 covering bins 1..1024
TT = 8          # t tiles of 128
LUT_W = 1088    # LUT row padded to 1088 f32 = 4352 B (multiple of 256)
NQ = 4          # SWDGE queues (ucode max)
MAX_POWER = 0.1

_CACHE = {}


def _build_lut() -> np.ndarray:
    """LUT[f0, k] = reference mask value at bin k given fundamental f0.

    Mirrors reference.py arithmetic in float32 exactly. f0=0 and f0>1020
    both yield the all-0.5 row (ok=False everywhere).
    """
    if "lut" in _CACHE:
        return _CACHE["lut"]
    k = np.arange(F, dtype=np.int64)[None, :]       # [1, k]
    f0 = np.arange(F, dtype=np.int64)[:, None]      # [f0, 1]
    f0safe = np.maximum(f0, 1)
    limit = F - 3 - 2  # F - FREQ_MARGIN - 2 = 1020
    m_mult = np.minimum((k + 3) // f0safe, limit // f0safe)
    i_last = m_mult * f0safe
    dist = np.abs(k - i_last).astype(np.float32)
    val = np.maximum(
        np.float32(1.0) - (np.float32(0.5) * dist) / np.float32(3.0),
        np.float32(0.5),
    )
    ok = (f0 > 0) & (i_last >= f0safe) & (i_last >= k - 3)
    lut = np.where(ok, val, np.float32(0.5)).astype(np.float32)  # [f0, k]
    out = np.full((F, LUT_W), 0.5, dtype=np.float32)
    out[:, :F] = lut
    _CACHE["lut"] = out
    return out


def _build_nc():
    if "nc" in _CACHE:
        return _CACHE["nc"]
    from contextlib import ExitStack

    nc = bacc.Bacc(
        "TRN2", target_bir_lowering=False, debug=False, num_swdge_queues=NQ
    )
    x_in = nc.dram_tensor("x", [NB, F, T], dt.float32, kind="ExternalInput").ap()
    lut_d = nc.dram_tensor("lut", [F, LUT_W], dt.float32, kind="ExternalInput").ap()
    ident_d = nc.dram_tensor("ident", [128, 128], dt.float32, kind="ExternalInput").ap()
    out_d = nc.dram_tensor("out", [NB, F, T], dt.float32, kind="ExternalOutput").ap()

    with TileContext(nc) as tc, ExitStack() as ctx:
        const_pool = ctx.enter_context(tc.tile_pool(name="constp", bufs=1))
        nat_pool = ctx.enter_context(tc.tile_pool(name="natp", bufs=9))
        xt_pool = ctx.enter_context(tc.tile_pool(name="xtp", bufs=10))
        gg_pool = ctx.enter_context(tc.tile_pool(name="ggp", bufs=10))
        gout_pool = ctx.enter_context(tc.tile_pool(name="goutp", bufs=4))
        psum_pool = ctx.enter_context(tc.tile_pool(name="psump", bufs=3, space="PSUM"))
        small_pool = ctx.enter_context(tc.tile_pool(name="smallp", bufs=2))

        ident_sb = const_pool.tile([128, 128], dt.float32, name="ident_sb")
        nc.sync.dma_start(ident_sb[:], ident_d[:])

        for b in range(NB):
            # ---- load + transpose to [t-part, f-free] ----
            xts = [
                xt_pool.tile([128, FT * 128], dt.float32, name=f"xt{b}_{g}", tag="xt")
                for g in range(TT)
            ]
            for ftg in range(2):  # two groups of 4 f-tiles
                nats = []
                for j in range(4):
                    ft = ftg * 4 + j
                    nat = nat_pool.tile([128, T], dt.float32, name=f"nat{b}_{ft}", tag="nat")
                    nc.sync.dma_start(
                        nat[:], x_in[b, 1 + ft * 128: 1 + (ft + 1) * 128, :]
                    )
                    nats.append(nat)
                for g in range(TT):
                    ps = psum_pool.tile(
                        [128, 512], dt.float32, name=f"ps{b}_{ftg}_{g}", tag="ps"
                    )
                    for j in range(4):
                        nc.tensor.transpose(
                            ps[:, j * 128:(j + 1) * 128],
                            nats[j][:, g * 128:(g + 1) * 128],
                            ident_sb[:],
                        )
                    nc.scalar.copy(xts[g][:, ftg * 512:(ftg + 1) * 512], ps[:])

            # ---- top-8 per column ----
            vals = small_pool.tile([128, 64], dt.float32, name=f"vals{b}", tag="vals")
            idxs = small_pool.tile([128, 64], dt.uint32, name=f"idxs{b}", tag="idxs")
            for g in range(TT):
                nc.vector.max(vals[:, 8 * g:8 * g + 8], xts[g][:])
                nc.vector.max_index(idxs[:, 8 * g:8 * g + 8], vals[:, 8 * g:8 * g + 8], xts[g][:])

            # ---- exact f0 (slot 0..4 = top-5; +1 bin offset; >0.1 gate) ----
            idxp = small_pool.tile([128, 64], dt.uint32, name=f"idxp{b}", tag="idxp")
            mask = small_pool.tile([128, 64], dt.uint32, name=f"mask{b}", tag="mask")
            cand = small_pool.tile([128, 64], dt.uint32, name=f"cand{b}", tag="cand")
            f0u = small_pool.tile([128, 8], dt.uint32, name=f"f0u{b}", tag="f0u")
            f0h = small_pool.tile([128, 8], dt.int16, name=f"f0h{b}", tag="f0h")
            nc.vector.tensor_scalar(idxp[:], idxs[:], 1, None, Alu.add)
            nc.vector.tensor_scalar(mask[:], vals[:], float(MAX_POWER), None, Alu.is_gt)
            nc.vector.memset(cand[:], 1024)
            nc.vector.copy_predicated(cand[:], mask[:], idxp[:])
            cand_v = cand[:].rearrange("p (g s) -> p g s", s=8)[:, :, 0:5]
            nc.vector.tensor_reduce(
                f0u[:], cand_v, axis=mybir.AxisListType.X, op=Alu.min
            )
            nc.vector.tensor_copy(f0h[:], f0u[:])

            # ---- wrapped int16 index list for dma_gather ----
            # wrapped[q, g*8+a] = f0(t = g*128 + a*16 + q) ; replicate to all
            # 16-partition groups (descriptor gen reads per-16-row groups).
            wrapped = small_pool.tile([128, 64], dt.int16, name=f"wrap{b}", tag="wrap")
            wv = wrapped[:].rearrange("p (g a) -> p g a", a=8)
            with nc.allow_non_contiguous_dma("tiny f0 index shuffle"):
                for a in range(8):
                    nc.scalar.dma_start(wv[0:16, :, a:a + 1], f0h[16 * a:16 * (a + 1), :])
            nc.scalar.dma_start(wrapped[16:32, :], wrapped[0:16, :])
            nc.scalar.dma_start(wrapped[32:64, :], wrapped[0:32, :])
            nc.scalar.dma_start(wrapped[64:128, :], wrapped[0:64, :])

            # ---- gather LUT rows (DMA engines) -> [t-part, k-free] tiles ----
            ggs = []
            for g in range(TT):
                gg = gg_pool.tile([128, LUT_W], dt.float32, name=f"gg{b}_{g}", tag="gg")
                nc.gpsimd.dma_gather(
                    gg[:].rearrange("p (c e) -> p c e", c=1),
                    lut_d[:],
                    wrapped[:, 8 * g:8 * g + 8],
                    num_idxs=128,
                    num_idxs_reg=128,
                    elem_size=LUT_W,
                    queue_num=(b * TT + g) % NQ,
                )
                ggs.append(gg)

            # ---- transpose back to natural [k-part, t-free] + write out ----
            for h in range(FT):
                out_nat = gout_pool.tile([128, T], dt.float32, name=f"on{b}_{h}", tag="onat")
                for half in range(2):
                    pso = psum_pool.tile(
                        [128, 512], dt.float32, name=f"pso{b}_{h}_{half}", tag="pso"
                    )
                    for j in range(4):
                        g = half * 4 + j
                        nc.tensor.transpose(
                            pso[:, j * 128:(j + 1) * 128],
                            ggs[g][:, h * 128:(h + 1) * 128],
                            ident_sb[:],
                        )
                    nc.scalar.copy(out_nat[:, half * 512:(half + 1) * 512], pso[:])
                nc.sync.dma_start(out_d[b, h * 128:(h + 1) * 128, :], out_nat[:])
            # k=1024 row: strided tiny DMAs straight from the gathered tiles
            with nc.allow_non_contiguous_dma("last output row"):
                for g in range(TT):
                    nc.sync.dma_start(
                        out_d[b, 1024:1025, g * 128:(g + 1) * 128],
                        ggs[g][:, 1024:1025],
                    )

    nc.compile()
    _CACHE["nc"] = nc
    return nc


def _make_in_maps(x: np.ndarray) -> list[dict]:
    lut = _build_lut()
    ident = np.eye(128, dtype=np.float32)
    return [
        {
            "x": np.ascontiguousarray(x[NB * c:NB * (c + 1), 0]),
            "lut": lut,
            "ident": ident,
        }
        for c in range(NCORES)
    ]


def kernel(x: np.ndarray) -> np.ndarray:
    x = np.asarray(x)
    assert x.shape == (B, 1, F, T), x.shape
    nc = _build_nc()
    in_maps = _make_in_maps(x)
    res = run_bass_kernel_spmd(nc, in_maps, core_ids=list(range(NCORES)))
    out = np.concatenate([res.results[c]["out"] for c in range(NCORES)], axis=0)
    return out[:, None, :, :].astype(np.float32, copy=False)



# revision 5
# speedup vs baseline: 1.0893x; 1.0893x over previous
"""Trainium2 Bass kernel for nn_HarmonicEstimation (topk_masking).

Problem: x [16,1,1025,1024] f32 -> mask [16,1,1025,1024].
Per (batch, t) column over f-bins 1..1024: find top-5 peaks, f0 = min index
among peaks with value > 0.1 (else 0); output column = harmonic-comb mask
that depends ONLY on f0.

Strategy (8 cores, 2 batches/core, no communication):
  - Output column is a pure function of f0; precompute LUT[f0, k] on host.
    Row k=1024 of the output is constant 0.5 (dist >= 4 for every f0), so
    the LUT only covers k=0..1023 -> rows of 1024 bf16 = 2048B. bf16
    rounding only touches the sparse comb bumps (rel err ~7e-4 overall).
  - Load input tiles in natural [f-part, t-free] layout (contiguous DMA),
    transpose to [t-part, f-free] via PE identity matmuls into a 2-bank
    PSUM tile; DVE max8/find_index8 read PSUM directly (no SBUF
    evacuation pass at all).
  - Tiny ops derive exact f0 per column (ties resolved like jax.lax.top_k);
    sentinel 1024 selects the all-0.5 LUT row.
  - ONE dma_gather(transpose=True) per batch pulls all 1024 LUT rows and
    delivers them TRANSPOSED: out[p, c, t] = LUT[f0_t][c*128+p], i.e.
    natural [k-part, t-free] tiles - no output transpose matmuls needed.
  - bf16->f32 convert per 128-row chunk (split across ACT/DVE/GPSIMD),
    contiguous DMA out; one extra 4KB DMA writes the constant k=1024 row.
"""

import os
import sys

for _p in ("/opt/trn_rl_repo", "/root/.axon_site/_ro/trn_rl_repo"):
    if os.path.isdir(_p) and _p not in sys.path:
        sys.path.insert(0, _p)

import numpy as np
import ml_dtypes

import concourse.bacc as bacc
import concourse.mybir as mybir
from concourse.bass_utils import run_bass_kernel_spmd
from concourse.tile import TileContext

dt = mybir.dt
Alu = mybir.AluOpType

B = 16          # full batch
NB = 2          # batches per core
NCORES = 8
F = 1025        # freq bins (0..1024)
T = 1024        # time columns
FT = 8          # f tiles of 128 covering bins 1..1024
TT = 8          # t tiles of 128
NQ = 4          # SWDGE queues
MAX_POWER = 0.1

_CACHE = {}


def _build_lut() -> np.ndarray:
    """LUT[f0, k] (k=0..1023) = reference mask value at bin k given f0,
    rounded to bf16. f0=0 and f0>1020 rows are all-0.5 (ok=False)."""
    if "lut" in _CACHE:
        return _CACHE["lut"]
    k = np.arange(1024, dtype=np.int64)[None, :]    # [1, k]
    f0 = np.arange(F, dtype=np.int64)[:, None]      # [f0, 1]
    f0safe = np.maximum(f0, 1)
    limit = F - 3 - 2  # 1020
    m_mult = np.minimum((k + 3) // f0safe, limit // f0safe)
    i_last = m_mult * f0safe
    dist = np.abs(k - i_last).astype(np.float32)
    val = np.maximum(
        np.float32(1.0) - (np.float32(0.5) * dist) / np.float32(3.0),
        np.float32(0.5),
    )
    ok = (f0 > 0) & (i_last >= f0safe) & (i_last >= k - 3)
    lut = np.where(ok, val, np.float32(0.5)).astype(ml_dtypes.bfloat16)
    _CACHE["lut"] = lut
    return lut


def _build_nc():
    if "nc" in _CACHE:
        return _CACHE["nc"]
    from contextlib import ExitStack

    nc = bacc.Bacc(
        "TRN2", target_bir_lowering=False, debug=False, num_swdge_queues=NQ
    )
    x_in = nc.dram_tensor("x", [NB, F, T], dt.float32, kind="ExternalInput").ap()
    lut_d = nc.dram_tensor("lut", [F, 1024], dt.bfloat16, kind="ExternalInput").ap()
    ident_d = nc.dram_tensor("ident", [128, 128], dt.float32, kind="ExternalInput").ap()
    out_d = nc.dram_tensor("out", [NB, F, T], dt.float32, kind="ExternalOutput").ap()

    with TileContext(nc) as tc, ExitStack() as ctx:
        const_pool = ctx.enter_context(tc.tile_pool(name="constp", bufs=1))
        nat_pool = ctx.enter_context(tc.tile_pool(name="natp", bufs=12))
        gg_pool = ctx.enter_context(tc.tile_pool(name="ggp", bufs=2))
        out_pool = ctx.enter_context(tc.tile_pool(name="outp", bufs=6))
        psum_pool = ctx.enter_context(tc.tile_pool(name="psump", bufs=4, space="PSUM"))
        small_pool = ctx.enter_context(tc.tile_pool(name="smallp", bufs=2))

        ident_sb = const_pool.tile([128, 128], dt.float32, name="ident_sb")
        nc.sync.dma_start(ident_sb[:], ident_d[:])
        hrow = const_pool.tile([1, T], dt.float32, name="hrow")
        nc.vector.memset(hrow[:], 0.5)

        for b in range(NB):
            # ---- load natural [f-part, t-free] tiles ----
            nats = []
            for ft in range(FT):
                nat = nat_pool.tile([128, T], dt.float32, name=f"nat{b}_{ft}", tag="nat")
                nc.sync.dma_start(
                    nat[:], x_in[b, 1 + ft * 128: 1 + (ft + 1) * 128, :]
                )
                nats.append(nat)

            # ---- per t-tile: transpose into 2-bank PSUM, top-8 off PSUM ----
            vals = small_pool.tile([128, 64], dt.float32, name=f"vals{b}", tag="vals")
            idxs = small_pool.tile([128, 64], dt.uint32, name=f"idxs{b}", tag="idxs")
            for g in range(TT):
                ps = psum_pool.tile([128, 1024], dt.float32, name=f"ps{b}_{g}", tag="ps")
                for ft in range(FT):
                    nc.tensor.transpose(
                        ps[:, ft * 128:(ft + 1) * 128],
                        nats[ft][:, g * 128:(g + 1) * 128],
                        ident_sb[:],
                    )
                nc.vector.max(vals[:, 8 * g:8 * g + 8], ps[:])
                nc.vector.max_index(idxs[:, 8 * g:8 * g + 8], vals[:, 8 * g:8 * g + 8], ps[:])

            # ---- exact f0 (slot 0..4 = top-5; +1 bin offset; >0.1 gate) ----
            idxp = small_pool.tile([128, 64], dt.uint32, name=f"idxp{b}", tag="idxp")
            mask = small_pool.tile([128, 64], dt.uint32, name=f"mask{b}", tag="mask")
            cand = small_pool.tile([128, 64], dt.uint32, name=f"cand{b}", tag="cand")
            f0u = small_pool.tile([128, 8], dt.uint32, name=f"f0u{b}", tag="f0u")
            f0h = small_pool.tile([128, 8], dt.int16, name=f"f0h{b}", tag="f0h")
            nc.vector.tensor_scalar(idxp[:], idxs[:], 1, None, Alu.add)
            nc.vector.tensor_scalar(mask[:], vals[:], float(MAX_POWER), None, Alu.is_gt)
            nc.vector.memset(cand[:], 1024)
            nc.vector.copy_predicated(cand[:], mask[:], idxp[:])
            cand_v = cand[:].rearrange("p (g s) -> p g s", s=8)[:, :, 0:5]
            nc.vector.tensor_reduce(
                f0u[:], cand_v, axis=mybir.AxisListType.X, op=Alu.min
            )
            nc.vector.tensor_copy(f0h[:], f0u[:])

            # ---- wrapped int16 index list for dma_gather ----
            # wrapped[p, s] = f0(t = s*16 + p) for p<16, replicated to 128.
            wrapped = small_pool.tile([128, 64], dt.int16, name=f"wrap{b}", tag="wrap")
            wv = wrapped[:].rearrange("p (g a) -> p g a", a=8)
            with nc.allow_non_contiguous_dma("tiny f0 index shuffle"):
                for a in range(8):
                    nc.scalar.dma_start(wv[0:16, :, a:a + 1], f0h[16 * a:16 * (a + 1), :])
            nc.scalar.dma_start(wrapped[16:32, :], wrapped[0:16, :])
            nc.scalar.dma_start(wrapped[32:64, :], wrapped[0:32, :])
            nc.scalar.dma_start(wrapped[64:128, :], wrapped[0:64, :])

            # ---- transpose-gather: LUT rows arrive as natural k-tiles ----
            # Split 1024 columns into 2x512-idx gathers (the SWDGE descriptor
            # ring rejects the 514-desc/dma load of a single 1024-idx
            # transpose gather; 258 fits). Separate queues so their ring
            # bookkeeping doesn't serialize.
            NG = 2          # gathers per batch
            GI = T // NG    # 512 columns per gather
            ggs = []
            for j in range(NG):
                gg = gg_pool.tile([128, FT * GI], dt.bfloat16, name=f"gg{b}_{j}", tag="gg")
                ggv = gg[:].rearrange("p (c e) -> p c e", e=GI)
                nc.gpsimd.dma_gather(
                    ggv,
                    lut_d[:],
                    wrapped[:, (GI // 16) * j:(GI // 16) * (j + 1)],
                    num_idxs=GI,
                    num_idxs_reg=GI,
                    elem_size=1024,
                    transpose=True,
                    queue_num=b * NG + j,
                )
                ggs.append(ggv)

            # ---- bf16 -> f32 convert + contiguous write-out ----
            for c in range(FT):
                outf = out_pool.tile([128, T], dt.float32, name=f"of{b}_{c}", tag="of")
                for j in range(NG):
                    if c % 2 == 0:
                        nc.scalar.copy(outf[:, GI * j:GI * (j + 1)], ggs[j][:, c, :])
                    else:
                        nc.vector.tensor_copy(outf[:, GI * j:GI * (j + 1)], ggs[j][:, c, :])
                nc.sync.dma_start(out_d[b, c * 128:(c + 1) * 128, :], outf[:])
            # constant k=1024 row
            nc.sync.dma_start(out_d[b, 1024:1025, :], hrow[:])

    nc.compile()
    _CACHE["nc"] = nc
    return nc


def _make_in_maps(x: np.ndarray) -> list[dict]:
    lut = _build_lut()
    ident = np.eye(128, dtype=np.float32)
    return [
        {
            "x": np.ascontiguousarray(x[NB * c:NB * (c + 1), 0]),
            "lut": lut,
            "ident": ident,
        }
        for c in range(NCORES)
    ]


def kernel(x: np.ndarray) -> np.ndarray:
    x = np.asarray(x)
    assert x.shape == (B, 1, F, T), x.shape
    nc = _build_nc()
    in_maps = _make_in_maps(x)
    res = run_bass_kernel_spmd(nc, in_maps, core_ids=list(range(NCORES)))
    out = np.concatenate([res.results[c]["out"] for c in range(NCORES)], axis=0)
    return out[:, None, :, :].astype(np.float32, copy=False)


# revision 12
# speedup vs baseline: 1.1481x; 1.0540x over previous
"""Trainium2 Bass kernel for nn_HarmonicEstimation (topk_masking).

Problem: x [16,1,1025,1024] f32 -> mask [16,1,1025,1024].
Per (batch, t) column over f-bins 1..1024: find top-5 peaks, f0 = min index
among peaks with value > 0.1 (else 0); output column = harmonic-comb mask
that depends ONLY on f0.

Strategy (8 cores, 2 batches/core, no communication):
  - Output column is a pure function of f0; precompute LUT on host. Row
    k=1024 of the output is constant 0.5 (dist >= 4 for every f0), so the
    LUT covers k=0..1023 -> rows of 1024 bf16. bf16 rounding only touches
    the sparse comb bumps (rel err ~7e-4 overall). The LUT is pre-shifted
    by one row (row r = mask for f0=r+1) so the raw find_index8 output
    indexes it directly; row 1024 is the all-0.5 row for "no valid peak".
  - Input loads in t-halves so the first transposed tile is ready early;
    PE identity matmuls transpose into 2-bank PSUM tiles; DVE
    max8/find_index8 read PSUM directly (no SBUF evacuation).
  - f0 post-processing runs on the Pool engine (4 fused ops) to keep DVE
    pure-throughput: cand = idx + (val<=0.1)*1024; f0 = min(min_5(cand),
    1024) cast to int16.
  - dma_gather(transpose=True) pulls LUT rows TRANSPOSED: chunks arrive as
    natural [k-part, t-free] tiles. Split 2x512 idx per batch (the SWDGE
    descriptor ring rejects a single 1024-idx transpose gather).
  - bf16->f32 convert per 128-row chunk (ACT/DVE alternate), contiguous
    DMA out; one 4KB DMA writes the constant k=1024 row.
  - Emission is phase-ordered across both batches so no engine's in-order
    queue has head-of-line blocking (b1's index shuffle must not sit
    behind b0's converts).
"""

import os
import sys

for _p in ("/opt/trn_rl_repo", "/root/.axon_site/_ro/trn_rl_repo"):
    if os.path.isdir(_p) and _p not in sys.path:
        sys.path.insert(0, _p)

import numpy as np
import ml_dtypes

import concourse.bacc as bacc
import concourse.mybir as mybir
from concourse.bass_utils import run_bass_kernel_spmd
from concourse.tile import TileContext

dt = mybir.dt
Alu = mybir.AluOpType

B = 16          # full batch
NB = 2          # batches per core
NCORES = 8
F = 1025        # freq bins (0..1024)
T = 1024        # time columns
FT = 8          # f tiles of 128 covering bins 1..1024
TT = 8          # t tiles of 128
NG = 2          # gathers per batch (512 idx each)
GI = T // NG
NQ = 4          # SWDGE queues
MAX_POWER = 0.1

_CACHE = {}


def _build_lut() -> np.ndarray:
    """LUT[r, k] (k=0..1023) = reference mask at bin k for f0 = r+1, bf16.
    Row 1024 = all-0.5 (selected by the no-valid-peak sentinel; f0 values
    1021..1024 are all-0.5 rows too, so the sentinel clamp is exact)."""
    if "lut" in _CACHE:
        return _CACHE["lut"]
    k = np.arange(1024, dtype=np.int64)[None, :]    # [1, k]
    f0 = np.arange(1, F + 1, dtype=np.int64)[:, None]  # rows for f0=1..1025
    limit = F - 3 - 2  # 1020
    m_mult = np.minimum((k + 3) // f0, limit // f0)
    i_last = m_mult * f0
    dist = np.abs(k - i_last).astype(np.float32)
    val = np.maximum(
        np.float32(1.0) - (np.float32(0.5) * dist) / np.float32(3.0),
        np.float32(0.5),
    )
    ok = (i_last >= f0) & (i_last >= k - 3)
    lut = np.where(ok, val, np.float32(0.5)).astype(ml_dtypes.bfloat16)
    _CACHE["lut"] = lut
    return lut


def _build_nc():
    if "nc" in _CACHE:
        return _CACHE["nc"]
    from contextlib import ExitStack

    nc = bacc.Bacc(
        "TRN2", target_bir_lowering=False, debug=False, num_swdge_queues=NQ
    )
    x_in = nc.dram_tensor("x", [NB, F, T], dt.float32, kind="ExternalInput").ap()
    lut_d = nc.dram_tensor("lut", [F, 1024], dt.bfloat16, kind="ExternalInput").ap()
    ident_d = nc.dram_tensor("ident", [128, 128], dt.float32, kind="ExternalInput").ap()
    out_d = nc.dram_tensor("out", [NB, F, T], dt.float32, kind="ExternalOutput").ap()

    with TileContext(nc) as tc, ExitStack() as ctx:
        const_pool = ctx.enter_context(tc.tile_pool(name="constp", bufs=1))
        nat_pool = ctx.enter_context(tc.tile_pool(name="natp", bufs=32))
        gg_pool = ctx.enter_context(tc.tile_pool(name="ggp", bufs=4))
        out_pool = ctx.enter_context(tc.tile_pool(name="outp", bufs=6))
        psum_pool = ctx.enter_context(tc.tile_pool(name="psump", bufs=4, space="PSUM"))
        small_pool = ctx.enter_context(tc.tile_pool(name="smallp", bufs=2))

        ident_sb = const_pool.tile([128, 128], dt.float32, name="ident_sb")
        nc.sync.dma_start(ident_sb[:], ident_d[:])
        hrow = const_pool.tile([1, T], dt.float32, name="hrow")
        nc.vector.memset(hrow[:], 0.5)

        # ---- phase 1: all input loads, t-halved for early first tile ----
        nats = {}
        for b in range(NB):
            for ft in range(FT):
                for h in range(2):
                    nat = nat_pool.tile(
                        [128, T // 2], dt.float32, name=f"nat{b}_{ft}_{h}", tag="nat"
                    )
                    nc.sync.dma_start(
                        nat[:],
                        x_in[b, 1 + ft * 128: 1 + (ft + 1) * 128,
                             h * (T // 2):(h + 1) * (T // 2)],
                    )
                    nats[(b, ft, h)] = nat

        # ---- phase 2 per batch: transpose+topk, f0, idx shuffle, gather ----
        ggs = {}
        for b in range(NB):
            vals = small_pool.tile([128, 64], dt.float32, name=f"vals{b}", tag="vals")
            idxs = small_pool.tile([128, 64], dt.uint32, name=f"idxs{b}", tag="idxs")
            for g in range(TT):
                h, gc = divmod(g, 4)
                ps = psum_pool.tile([128, 1024], dt.float32, name=f"ps{b}_{g}", tag="ps")
                for ft in range(FT):
                    nc.tensor.transpose(
                        ps[:, ft * 128:(ft + 1) * 128],
                        nats[(b, ft, h)][:, gc * 128:(gc + 1) * 128],
                        ident_sb[:],
                    )
                nc.vector.max(vals[:, 8 * g:8 * g + 8], ps[:])
                nc.vector.max_index(idxs[:, 8 * g:8 * g + 8], vals[:, 8 * g:8 * g + 8], ps[:])

            # f0 chain (4 DVE ops, high-priority so the scheduler doesn't
            # wedge b1 scans into it): cand = idx | (val<=0.1)*2048 (bit 11
            # marks invalid; idx <= 1023 so OR == add); f0 = min of the 5
            # top-k slots clamped to 1024 (LUT is 1-row-shifted so the raw
            # find_index8 output indexes it; 1024 = all-0.5 sentinel row).
            inv = small_pool.tile([128, 64], dt.uint32, name=f"inv{b}", tag="inv")
            cand = small_pool.tile([128, 64], dt.uint32, name=f"cand{b}", tag="cand")
            f0u = small_pool.tile([128, 8], dt.uint32, name=f"f0u{b}", tag="f0u")
            f0h = small_pool.tile([128, 8], dt.int16, name=f"f0h{b}", tag="f0h")
            with tc.high_priority():
                nc.vector.tensor_scalar(
                    inv[:], vals[:], float(MAX_POWER), 2048, Alu.is_le, Alu.mult
                )
                nc.vector.tensor_tensor(cand[:], idxs[:], inv[:], Alu.bitwise_or)
                cand_v = cand[:].rearrange("p (g s) -> p g s", s=8)[:, :, 0:5]
                nc.vector.tensor_reduce(
                    f0u[:], cand_v, axis=mybir.AxisListType.X, op=Alu.min
                )
                nc.vector.tensor_scalar(f0h[:], f0u[:], 1024, None, Alu.min)

            # wrapped[p, s] = f0(t = s*16 + p) for p<16, replicated to 128.
            # 8 shuffle DMAs split across ACT and SP queues.
            wrapped = small_pool.tile([128, 64], dt.int16, name=f"wrap{b}", tag="wrap")
            wv = wrapped[:].rearrange("p (g a) -> p g a", a=8)
            with nc.allow_non_contiguous_dma("tiny f0 index shuffle"):
                for a in range(8):
                    eng = nc.scalar if a % 2 == 0 else nc.sync
                    eng.dma_start(wv[0:16, :, a:a + 1], f0h[16 * a:16 * (a + 1), :])
            nc.scalar.dma_start(wrapped[16:32, :], wrapped[0:16, :])
            nc.scalar.dma_start(wrapped[32:64, :], wrapped[0:32, :])
            nc.scalar.dma_start(wrapped[64:128, :], wrapped[0:64, :])

            # transpose-gather: LUT rows arrive as natural k-tiles.
            for j in range(NG):
                gg = gg_pool.tile([128, FT * GI], dt.bfloat16, name=f"gg{b}_{j}", tag="gg")
                ggv = gg[:].rearrange("p (c e) -> p c e", e=GI)
                nc.gpsimd.dma_gather(
                    ggv,
                    lut_d[:],
                    wrapped[:, (GI // 16) * j:(GI // 16) * (j + 1)],
                    num_idxs=GI,
                    num_idxs_reg=GI,
                    elem_size=1024,
                    transpose=True,
                    queue_num=b * NG + j,
                )
                ggs[(b, j)] = ggv

        # ---- phase 3 per batch: bf16 -> f32 convert + write-out ----
        for b in range(NB):
            for c in range(FT):
                outf = out_pool.tile([128, T], dt.float32, name=f"of{b}_{c}", tag="of")
                for j in range(NG):
                    if c % 2 == 0:
                        nc.scalar.copy(outf[:, GI * j:GI * (j + 1)], ggs[(b, j)][:, c, :])
                    else:
                        nc.vector.tensor_copy(outf[:, GI * j:GI * (j + 1)], ggs[(b, j)][:, c, :])
                nc.sync.dma_start(out_d[b, c * 128:(c + 1) * 128, :], outf[:])
            nc.sync.dma_start(out_d[b, 1024:1025, :], hrow[:])

    nc.compile()
    _CACHE["nc"] = nc
    return nc


def _make_in_maps(x: np.ndarray) -> list[dict]:
    lut = _build_lut()
    ident = np.eye(128, dtype=np.float32)
    return [
        {
            "x": np.ascontiguousarray(x[NB * c:NB * (c + 1), 0]),
            "lut": lut,
            "ident": ident,
        }
        for c in range(NCORES)
    ]


def kernel(x: np.ndarray) -> np.ndarray:
    x = np.asarray(x)
    assert x.shape == (B, 1, F, T), x.shape
    nc = _build_nc()
    in_maps = _make_in_maps(x)
    res = run_bass_kernel_spmd(nc, in_maps, core_ids=list(range(NCORES)))
    out = np.concatenate([res.results[c]["out"] for c in range(NCORES)], axis=0)
    return out[:, None, :, :].astype(np.float32, copy=False)
